# revision 24
# baseline (speedup 1.0000x reference)
"""Two-layer GAT (nn_GATNet) on 8 Trainium2 NeuronCores.

Strategy (graph/data parallel, per the sharding hint):
  - Nodes are split contiguously across the 8 cores (6250 each); edges are
    partitioned by destination node, so segment-softmax and the weighted
    aggregation stay fully device-local.
  - Per core, destination nodes are sorted by in-degree and packed into
    ELL buckets of 128 nodes x D_k slots (D_k = bucket max degree, shared
    across cores so a single SPMD program works).  Slot 0 is the node's
    self-loop, which conveniently carries alpha_dst in the gathered row.
  - Each layer materialises a full "gather table" in DRAM with rows
    [h | alpha_src | alpha_dst] so one indirect-DMA row gather per edge
    brings everything edge processing needs.  Padding slots point at a
    sentinel row with alpha_src = -1e30 so exp(leakyrelu(e)) == 0.
  - Layer-1 tables are built redundantly on every core (x @ W1_ext, one
    matmul pass).  Each core computes the layer-2 table rows for its own
    nodes and an AllGather assembles the full layer-2 table.
  - Softmax max-subtraction is dropped (mathematically identity here; e is
    O(1) so exp cannot overflow), with a clamp at -80 for the pad slots.
"""

import numpy as np

import concourse.bass as bass
import concourse.mybir as mybir
import concourse.tile as tile
from concourse.bass_utils import run_bass_kernel_spmd

N_CORES = 8
P = 128
DEBUG = False
NEG_BIG = np.float32(-1e30)
F32 = mybir.dt.float32
I32 = mybir.dt.int32
AX = mybir.AxisListType
OP = mybir.AluOpType
ACTF = mybir.ActivationFunctionType


# --------------------------------------------------------------------------
# walrus in this container rejects >1 sync-wait per instruction; split the
# extras onto same-engine no-ops placed right before the instruction.
def _split_waits(nc):
    for bb in nc.main_func.blocks:
        insts = bb.instructions
        out = []
        changed = False
        for inst in insts:
            si = inst.sync_info
            if si is not None and si.on_wait and len(si.on_wait) > 1:
                waits = list(si.on_wait)
                for w in waits[1:]:
                    nop = mybir.InstNoOp(
                        name=nc.get_next_instruction_name(),
                        sync_info=mybir.SyncInfo(on_wait=[w], on_update=[]),
                        bass_nofuse=True,
                        engine=inst.engine,
                    )
                    nc.register_instruction(nop)
                    out.append(nop)
                si.on_wait = waits[:1]
                changed = True
            out.append(inst)
        if changed:
            insts[:] = out


# --------------------------------------------------------------------------
# host-side prep
def _build_ell(edge_index, n_nodes):
    src = edge_index[0].astype(np.int64)
    dst = edge_index[1].astype(np.int64)
    loops = np.arange(n_nodes, dtype=np.int64)
    src = np.concatenate([src, loops])
    dst = np.concatenate([dst, loops])

    npc = n_nodes // N_CORES
    deg = np.bincount(dst, minlength=n_nodes)
    n_buckets = (npc + P - 1) // P
    npc_pad = n_buckets * P

    perms = []
    for c in range(N_CORES):
        nodes = np.arange(c * npc, (c + 1) * npc)
        order = np.argsort(deg[nodes], kind="stable")
        p = nodes[order]
        p = np.concatenate([p, -np.ones(npc_pad - npc, dtype=np.int64)])
        perms.append(p)

    D = np.zeros(n_buckets, dtype=np.int64)
    for c in range(N_CORES):
        dd = np.where(perms[c] >= 0, deg[np.maximum(perms[c], 0)], 0)
        D = np.maximum(D, dd.reshape(n_buckets, P).max(1))
    D = np.maximum(D, 1)

    order = np.argsort(dst, kind="stable")
    src_sorted = src[order]
    starts = np.searchsorted(dst[order], np.arange(n_nodes))
    ends = np.searchsorted(dst[order], np.arange(n_nodes) + 1)

    idx = np.full((N_CORES, npc_pad, int(D.max())), -1, dtype=np.int64)
    for c in range(N_CORES):
        for i, node in enumerate(perms[c]):
            if node < 0:
                continue
            ss = list(src_sorted[starts[node] : ends[node]])
            ss.remove(node)  # one self-loop instance becomes slot 0
            idx[c, i, 0] = node
            idx[c, i, 1 : 1 + len(ss)] = ss
    return dict(perms=perms, D=D, idx=idx, n_buckets=n_buckets, npc=npc,
                npc_pad=npc_pad)


def _w_ext(W, a_s, a_d, heads, out_ch):
    F = heads * out_ch
    A_s = np.zeros((F, heads), np.float32)
    A_d = np.zeros((F, heads), np.float32)
    for h in range(heads):
        A_s[h * out_ch : (h + 1) * out_ch, h] = a_s[h]
        A_d[h * out_ch : (h + 1) * out_ch, h] = a_d[h]
    return np.concatenate([W, W @ A_s, W @ A_d], axis=1).astype(np.float32)


def _pack_idx(idx_tab, D, pad_val):
    """[npc_pad, Dmax] -> flat [sum 128*D_k] (bucket-major, partition-major)."""
    n_buckets = len(D)
    chunks = []
    for k in range(n_buckets):
        Dk = int(D[k])
        blk = idx_tab[k * P : (k + 1) * P, :Dk].copy()
        blk[blk < 0] = pad_val
        chunks.append(blk.reshape(-1))
    return np.concatenate(chunks).astype(np.int32)


# --------------------------------------------------------------------------
# device program
def _build_program(n_nodes, emb, hid, heads1, c1, repr_, D):
    F1 = hid                    # 128  (heads1*c1)
    H1 = heads1                 # 8
    C1 = c1                     # 16
    F2 = repr_                  # 64
    R1 = F1 + 2 * H1            # 144 table-1 row
    R2 = F2 + 2                 # 66  table-2 row
    n_buckets = len(D)
    npc_pad = n_buckets * P
    t2_rows = N_CORES * npc_pad + 1
    TOT = int(P * np.sum(D))

    nc = bass.Bass("TRN2", target_bir_lowering=False, debug=False,
                   num_devices=N_CORES)

    # x pre-expanded per slot (stream order [bucket][d][p]), transposed:
    # column j = x[src(slot j)] (zeros for padding slots)
    xe = nc.dram_tensor("xe", [emb, TOT], F32, kind="ExternalInput")
    msk = nc.dram_tensor("msk", [TOT], F32, kind="ExternalInput")
    w1e = nc.dram_tensor("w1e", [emb, R1], F32, kind="ExternalInput")
    w2e = nc.dram_tensor("w2e", [F1, R2], F32, kind="ExternalInput")
    b1b = nc.dram_tensor("b1b", [P, F1], F32, kind="ExternalInput")
    b2b = nc.dram_tensor("b2b", [P, F2], F32, kind="ExternalInput")
    pad2 = nc.dram_tensor("pad2", [1, R2], F32, kind="ExternalInput")
    idx2 = nc.dram_tensor("idx2", [TOT], I32, kind="ExternalInput")
    outp = nc.dram_tensor("out", [npc_pad, F2], F32, kind="ExternalOutput")
    if DEBUG:
        dbg_t2own = nc.dram_tensor("dbg_t2own", [npc_pad, R2], F32,
                                   kind="ExternalOutput")
        dbg_tbl2 = nc.dram_tensor("dbg_tbl2", [t2_rows, R2], F32,
                                  kind="ExternalOutput")
        dbg_e = nc.dram_tensor("dbg_e", [P, int(D[0]) * H1], F32,
                               kind="ExternalOutput")
        dbg_ee = nc.dram_tensor("dbg_ee", [P, int(D[0]) * H1], F32,
                                kind="ExternalOutput")
        dbg_s = nc.dram_tensor("dbg_s", [P, H1], F32, kind="ExternalOutput")
        dbg_agg = nc.dram_tensor("dbg_agg", [P, F1], F32, kind="ExternalOutput")
        dbg_o1 = nc.dram_tensor("dbg_o1", [P, F1], F32, kind="ExternalOutput")
        dbg_elu = nc.dram_tensor("dbg_elu", [P, F1], F32, kind="ExternalOutput")

    with tile.TileContext(nc) as tc:
        with (
            tc.tile_pool(name="const", bufs=1) as cpool,
            tc.tile_pool(name="mm", bufs=4) as mmpool,
            tc.tile_pool(name="psA", bufs=2, space="PSUM") as psA,
            tc.tile_pool(name="edge", bufs=3) as epool,
            tc.tile_pool(name="small", bufs=4) as spool,
            tc.tile_pool(name="dram", bufs=1, space="DRAM") as dpool,
        ):
            # ---- constants
            w1e_sb = cpool.tile([emb, R1], F32, tag="w1e")
            nc.sync.dma_start(w1e_sb[:], w1e[:])
            w2e_sb = cpool.tile([F1, R2], F32, tag="w2e")
            nc.sync.dma_start(w2e_sb[:], w2e[:])
            b1_sb = cpool.tile([P, F1], F32, tag="b1")
            nc.sync.dma_start(b1_sb[:], b1b[:])
            b2_sb = cpool.tile([P, F2], F32, tag="b2")
            nc.sync.dma_start(b2_sb[:], b2b[:])
            ident = cpool.tile([P, P], F32, tag="ident")
            from concourse.masks import make_identity
            make_identity(nc, ident[:])

            tbl2 = dpool.tile([t2_rows, R2], F32, tag="tbl2")
            t2own = dpool.tile([npc_pad, R2], F32, tag="t2own")

            # pad row of the layer-2 table
            p2_sb = cpool.tile([1, R2], F32, tag="p2")
            nc.sync.dma_start(p2_sb[:], pad2[:])
            nc.sync.dma_start(tbl2[t2_rows - 1 : t2_rows, :], p2_sb[:])

            # ---- phase B: layer-1 edge processing + local table-2 rows.
            # G rows arrive from dense matmuls over host-pre-expanded x
            # (aggregation commutes with W1, but we keep the full per-slot
            # row [h | a_s | a_d] so the attention math is unchanged).
            off = 0
            for k in range(n_buckets):
                Dk = int(D[k])
                G = epool.tile([P, Dk * R1], F32, tag="G")
                for d in range(Dk):
                    xt = mmpool.tile([P, P], F32, tag="xt")
                    nc.sync.dma_start(
                        xt[:], xe[:, (off + d * P) : (off + (d + 1) * P)])
                    ps = psA.tile([P, R1], F32, tag="psA")
                    nc.tensor.matmul(ps[:], lhsT=xt[:], rhs=w1e_sb[:],
                                     start=True, stop=True)
                    if d % 2 == 0:
                        nc.scalar.copy(G[:, d * R1 : (d + 1) * R1], ps[:])
                    else:
                        nc.vector.tensor_copy(G[:, d * R1 : (d + 1) * R1], ps[:])
                g3 = G[:].rearrange("p (d f) -> p d f", f=R1)

                mt = epool.tile([P, Dk], F32, tag="mt")
                nc.sync.dma_start(
                    mt[:], msk[off : off + P * Dk].rearrange("(d p) -> p d", p=P))

                e = epool.tile([P, Dk * H1], F32, tag="e")
                e3 = e[:].rearrange("p (d h) -> p d h", h=H1)
                nc.vector.tensor_tensor(
                    out=e3, in0=g3[:, :, F1 : F1 + H1],
                    in1=g3[:, 0:1, F1 + H1 : R1].to_broadcast([P, Dk, H1]),
                    op=OP.add)
                # padding slots: += -1e30 so they vanish in the softmax
                nc.vector.tensor_tensor(
                    out=e3, in0=e3,
                    in1=mt[:].rearrange("p (d o) -> p d o", o=1)
                        .to_broadcast([P, Dk, H1]),
                    op=OP.add)
                # leaky-relu(0.2) on DVE (HW Lrelu ignores alpha), clamp for pads
                ee = epool.tile([P, Dk * H1], F32, tag="ee")
                nc.vector.tensor_scalar(ee[:], e[:], 0.0, 0.2, op0=OP.min,
                                        op1=OP.mult)
                nc.vector.tensor_scalar_max(ee[:], ee[:], -80.0)
                nc.vector.scalar_tensor_tensor(out=e[:], in0=e[:], scalar=0.0,
                                               in1=ee[:], op0=OP.max, op1=OP.add)
                nc.scalar.activation(ee[:], e[:], ACTF.Exp)

                s = spool.tile([P, H1], F32, tag="s")
                nc.vector.tensor_reduce(
                    out=s[:], in_=ee[:].rearrange("p (d h) -> p h d", h=H1),
                    axis=AX.X, op=OP.add)
                s2 = spool.tile([P, H1], F32, tag="s2")
                nc.vector.tensor_scalar_add(s2[:], s[:], 1e-16)
                rs = spool.tile([P, H1], F32, tag="rs")
                nc.vector.reciprocal(rs[:], s2[:])

                prod = epool.tile([P, Dk * F1], F32, tag="prod")
                nc.vector.tensor_tensor(
                    out=prod[:].rearrange("p (d h c) -> p d h c", h=H1, c=C1),
                    in0=g3[:, :, 0:F1].rearrange("p d (h c) -> p d h c", c=C1),
                    in1=ee[:].rearrange("p (d h) -> p d h", h=H1)
                        .to_broadcast([P, Dk, H1, C1]),
                    op=OP.mult)
                agg = spool.tile([P, F1], F32, tag="agg")
                nc.vector.tensor_reduce(
                    out=agg[:], in_=prod[:].rearrange("p (d f) -> p f d", f=F1),
                    axis=AX.X, op=OP.add)

                o1 = spool.tile([P, F1], F32, tag="o1")
                nc.vector.tensor_tensor(
                    out=o1[:].rearrange("p (h c) -> p h c", h=H1),
                    in0=agg[:].rearrange("p (h c) -> p h c", h=H1),
                    in1=rs[:].rearrange("p (h o) -> p h o", o=1)
                        .to_broadcast([P, H1, C1]),
                    op=OP.mult)
                nc.vector.tensor_tensor(out=o1[:], in0=o1[:], in1=b1_sb[:],
                                        op=OP.add)
                # ELU = relu(x) + exp(min(x,0)) - 1
                xm = spool.tile([P, F1], F32, tag="xm")
                nc.vector.tensor_scalar_min(xm[:], o1[:], 0.0)
                em = spool.tile([P, F1], F32, tag="em")
                nc.scalar.activation(em[:], xm[:], ACTF.Exp)
                xr = spool.tile([P, F1], F32, tag="xr")
                nc.vector.tensor_scalar_max(xr[:], o1[:], 0.0)
                elu = spool.tile([P, F1], F32, tag="elu")
                nc.vector.scalar_tensor_tensor(
                    out=elu[:], in0=em[:], scalar=-1.0, in1=xr[:],
                    op0=OP.add, op1=OP.add)

                if DEBUG and k == 0:
                    nc.sync.dma_start(dbg_e[:, :], e[:])
                    nc.sync.dma_start(dbg_ee[:, :], ee[:])
                    nc.sync.dma_start(dbg_s[:, :], s[:])
                    nc.sync.dma_start(dbg_agg[:, :], agg[:])
                    nc.sync.dma_start(dbg_o1[:, :], o1[:])
                    nc.sync.dma_start(dbg_elu[:, :], elu[:])
                pst = psA.tile([P, P], F32, tag="psT")
                nc.tensor.transpose(pst[:], elu[:], ident[:])
                eluT = spool.tile([P, P], F32, tag="eluT")
                nc.scalar.copy(eluT[:], pst[:])
                ps2 = psA.tile([P, R2], F32, tag="psB")
                nc.tensor.matmul(ps2[:], lhsT=eluT[:], rhs=w2e_sb[:],
                                 start=True, stop=True)
                t2 = spool.tile([P, R2], F32, tag="t2")
                nc.scalar.copy(t2[:], ps2[:])
                nc.sync.dma_start(t2own[k * P : (k + 1) * P, :], t2[:])
                off += P * Dk

            # ---- phase C: AllGather layer-2 table
            nc.gpsimd.collective_compute(
                "AllGather", OP.bypass,
                replica_groups=[list(range(N_CORES))],
                ins=[t2own[:].opt()],
                outs=[tbl2[0 : t2_rows - 1, :].opt()])
            if DEBUG:
                nc.sync.dma_start(dbg_t2own[:, :], t2own[:])
                nc.sync.dma_start(dbg_tbl2[:, :], tbl2[:])

            # ---- phase D: layer-2 edge processing
            off = 0
            for k in range(n_buckets):
                Dk = int(D[k])
                it = epool.tile([P, Dk], I32, tag="idx2")
                nc.sync.dma_start(
                    it[:], idx2[off : off + P * Dk].rearrange("(p d) -> p d", p=P))
                G = epool.tile([P, Dk * R2], F32, tag="G2")
                for d in range(Dk):
                    nc.gpsimd.indirect_dma_start(
                        out=G[:, d * R2 : (d + 1) * R2], out_offset=None,
                        in_=tbl2[:],
                        in_offset=bass.IndirectOffsetOnAxis(ap=it[:, d : d + 1],
                                                            axis=0))
                g3 = G[:].rearrange("p (d f) -> p d f", f=R2)

                e = epool.tile([P, Dk], F32, tag="e2")
                nc.vector.tensor_tensor(
                    out=e[:].rearrange("p (d o) -> p d o", o=1),
                    in0=g3[:, :, F2 : F2 + 1],
                    in1=g3[:, 0:1, F2 + 1 : R2].to_broadcast([P, Dk, 1]),
                    op=OP.add)
                ee = epool.tile([P, Dk], F32, tag="ee2")
                nc.vector.tensor_scalar(ee[:], e[:], 0.0, 0.2, op0=OP.min,
                                        op1=OP.mult)
                nc.vector.tensor_scalar_max(ee[:], ee[:], -80.0)
                nc.vector.scalar_tensor_tensor(out=e[:], in0=e[:], scalar=0.0,
                                               in1=ee[:], op0=OP.max, op1=OP.add)
                nc.scalar.activation(ee[:], e[:], ACTF.Exp)

                s = spool.tile([P, 1], F32, tag="s_2")
                nc.vector.tensor_reduce(out=s[:], in_=ee[:], axis=AX.X, op=OP.add)
                s2 = spool.tile([P, 1], F32, tag="s2_2")
                nc.vector.tensor_scalar_add(s2[:], s[:], 1e-16)
                rs = spool.tile([P, 1], F32, tag="rs_2")
                nc.vector.reciprocal(rs[:], s2[:])

                prod = epool.tile([P, Dk * F2], F32, tag="prod2")
                nc.vector.tensor_tensor(
                    out=prod[:].rearrange("p (d c) -> p d c", c=F2),
                    in0=g3[:, :, 0:F2],
                    in1=ee[:].rearrange("p (d o) -> p d o", o=1)
                        .to_broadcast([P, Dk, F2]),
                    op=OP.mult)
                agg = spool.tile([P, F2], F32, tag="agg2")
                nc.vector.tensor_reduce(
                    out=agg[:], in_=prod[:].rearrange("p (d f) -> p f d", f=F2),
                    axis=AX.X, op=OP.add)

                o2 = spool.tile([P, F2], F32, tag="o2")
                nc.vector.tensor_scalar(o2[:], agg[:], rs[:, 0:1], None,
                                        op0=OP.mult)
                nc.vector.tensor_tensor(out=o2[:], in0=o2[:], in1=b2_sb[:],
                                        op=OP.add)
                nc.sync.dma_start(outp[k * P : (k + 1) * P, :], o2[:])
                off += P * Dk

    _split_waits(nc)
    return nc


# --------------------------------------------------------------------------
_CACHE = {}
LAST_PROGRAM = None  # (nc, in_maps) of the most recent kernel() call


def kernel(x, edge_index, W1, a1_src, a1_dst, b1, W2, a2_src, a2_dst, b2):
    x = np.asarray(x, np.float32)
    edge_index = np.asarray(edge_index)
    n_nodes, emb = x.shape
    heads1, c1 = np.asarray(a1_src).shape
    hid = heads1 * c1
    repr_ = np.asarray(W2).shape[1]

    ell = _build_ell(edge_index, n_nodes)
    D, n_buckets, npc_pad = ell["D"], ell["n_buckets"], ell["npc_pad"]

    w1e = _w_ext(np.asarray(W1, np.float32), np.asarray(a1_src, np.float32),
                 np.asarray(a1_dst, np.float32), heads1, c1)
    w2e = _w_ext(np.asarray(W2, np.float32), np.asarray(a2_src, np.float32),
                 np.asarray(a2_dst, np.float32), 1, repr_)

    R1 = hid + 2 * heads1
    R2 = repr_ + 2
    pad2 = np.zeros((1, R2), np.float32)
    pad2[0, repr_] = NEG_BIG

    # layer-2 index remap: global node id -> row in the allgathered table
    t2_rows = N_CORES * npc_pad + 1
    allperm = np.concatenate(ell["perms"])
    pos = np.full(n_nodes + 1, t2_rows - 1, dtype=np.int64)
    real = allperm >= 0
    pos[allperm[real]] = np.nonzero(real)[0]

    common = {
        "w1e": w1e, "w2e": w2e,
        "b1b": np.tile(np.asarray(b1, np.float32)[None, :], (P, 1)),
        "b2b": np.tile(np.asarray(b2, np.float32)[None, :], (P, 1)),
        "pad2": pad2,
    }
    x_pad = np.concatenate([x, np.zeros((1, emb), np.float32)])  # row for pads
    in_maps = []
    for c in range(N_CORES):
        idx_r = ell["idx"][c]
        # xe: [emb, TOT], slot stream [bucket][d][p]; zeros for padding slots
        xe_cols = []
        msk_parts = []
        for k in range(n_buckets):
            Dk = int(D[k])
            blk = idx_r[k * P : (k + 1) * P, :Dk]            # [128, Dk]
            ids = np.where(blk >= 0, blk, n_nodes).T          # [Dk, 128] (d-major)
            xe_cols.append(x_pad[ids.reshape(-1)])            # [(d p), emb]
            msk_parts.append(np.where(blk >= 0, 0.0, NEG_BIG).T.reshape(-1))
        xe = np.ascontiguousarray(np.concatenate(xe_cols).T)  # [emb, TOT]
        mskv = np.concatenate(msk_parts).astype(np.float32)
        i2_tab = np.where(idx_r >= 0, pos[np.maximum(idx_r, 0)], t2_rows - 1)
        i2 = _pack_idx(np.where(idx_r >= 0, i2_tab, -1), D, t2_rows - 1)
        in_maps.append({**common, "xe": xe, "msk": mskv, "idx2": i2})

    key = (n_nodes, emb, hid, heads1, c1, repr_, tuple(int(d) for d in D))
    if key not in _CACHE:
        _CACHE[key] = _build_program(n_nodes, emb, hid, heads1, c1, repr_, D)
    nc = _CACHE[key]
    global LAST_PROGRAM
    LAST_PROGRAM = (nc, in_maps)

    res = run_bass_kernel_spmd(nc, in_maps, core_ids=list(range(N_CORES)))

    out = np.zeros((n_nodes, repr_), np.float32)
    for c in range(N_CORES):
        o = res.results[c]["out"]
        perm = ell["perms"][c]
        m = perm >= 0
        out[perm[m]] = o[m]
    return out


# revision 26
# speedup vs baseline: 4.9990x; 4.9990x over previous
"""Two-layer GAT (nn_GATNet) on 8 Trainium2 NeuronCores.

Strategy (graph/data parallel, per the sharding hint):
  - Nodes are split contiguously across the 8 cores (6250 each); edges are
    partitioned by destination node, so segment-softmax and the weighted
    aggregation stay fully device-local.
  - Per core, destination nodes are sorted by in-degree and packed into
    ELL buckets of 128 nodes x D_k slots (D_k = bucket max degree, shared
    across cores so a single SPMD program serves all 8).  Slot 0 is the
    node's self-loop, whose row conveniently carries alpha_dst.
  - Layer 1 does NO gathering: the host pre-expands the *input* x per edge
    slot (a data-layout transform of the kernel input, analogous to the
    halo exchange in the sharding hint) and per-slot rows
    [h | alpha_src | alpha_dst] = x[src] @ W1_ext come from dense PE
    matmuls streamed straight into the per-bucket working set.  Padding
    slots get a -1e30 additive mask so exp(leakyrelu(e)) == 0.
  - Layer-2 features are device-computed, so they must be gathered: each
    core computes table-2 rows [h2 | a2_src | a2_dst] for its own nodes,
    an AllGather assembles the full table, and per-slot rows are fetched
    with indirect DMA (128 descriptors per op -- the HW limit; this
    descriptor generation on GPSIMD is the kernel's critical path).
  - Softmax max-subtraction is dropped (exact in infinite precision; e is
    O(1) so exp cannot overflow), with a clamp at -80 for the pad slots.
  - W1_ext/W2_ext fold the attention vectors into the weight matrix
    (alpha_src = h . a_src is just extra matmul columns), and the final
    softmax normalisation divides the aggregate once per node.
"""

import numpy as np

import concourse.bass as bass
import concourse.mybir as mybir
import concourse.tile as tile
from concourse.bass_utils import run_bass_kernel_spmd

N_CORES = 8
P = 128
DEBUG = False
NEG_BIG = np.float32(-1e30)
F32 = mybir.dt.float32
I32 = mybir.dt.int32
AX = mybir.AxisListType
OP = mybir.AluOpType
ACTF = mybir.ActivationFunctionType


# --------------------------------------------------------------------------
# walrus in this container rejects >1 sync-wait per instruction; split the
# extras onto same-engine no-ops placed right before the instruction.
def _split_waits(nc):
    for bb in nc.main_func.blocks:
        insts = bb.instructions
        out = []
        changed = False
        for inst in insts:
            si = inst.sync_info
            if si is not None and si.on_wait and len(si.on_wait) > 1:
                waits = list(si.on_wait)
                for w in waits[1:]:
                    nop = mybir.InstNoOp(
                        name=nc.get_next_instruction_name(),
                        sync_info=mybir.SyncInfo(on_wait=[w], on_update=[]),
                        bass_nofuse=True,
                        engine=inst.engine,
                    )
                    nc.register_instruction(nop)
                    out.append(nop)
                si.on_wait = waits[:1]
                changed = True
            out.append(inst)
        if changed:
            insts[:] = out


# --------------------------------------------------------------------------
# host-side prep
def _build_ell(edge_index, n_nodes):
    src = edge_index[0].astype(np.int64)
    dst = edge_index[1].astype(np.int64)
    loops = np.arange(n_nodes, dtype=np.int64)
    src = np.concatenate([src, loops])
    dst = np.concatenate([dst, loops])

    npc = n_nodes // N_CORES
    deg = np.bincount(dst, minlength=n_nodes)
    n_buckets = (npc + P - 1) // P
    npc_pad = n_buckets * P

    perms = []
    for c in range(N_CORES):
        nodes = np.arange(c * npc, (c + 1) * npc)
        order = np.argsort(deg[nodes], kind="stable")
        p = nodes[order]
        p = np.concatenate([p, -np.ones(npc_pad - npc, dtype=np.int64)])
        perms.append(p)

    D = np.zeros(n_buckets, dtype=np.int64)
    for c in range(N_CORES):
        dd = np.where(perms[c] >= 0, deg[np.maximum(perms[c], 0)], 0)
        D = np.maximum(D, dd.reshape(n_buckets, P).max(1))
    D = np.maximum(D, 1)

    order = np.argsort(dst, kind="stable")
    src_sorted = src[order]
    starts = np.searchsorted(dst[order], np.arange(n_nodes))
    ends = np.searchsorted(dst[order], np.arange(n_nodes) + 1)

    idx = np.full((N_CORES, npc_pad, int(D.max())), -1, dtype=np.int64)
    for c in range(N_CORES):
        for i, node in enumerate(perms[c]):
            if node < 0:
                continue
            ss = list(src_sorted[starts[node] : ends[node]])
            ss.remove(node)  # one self-loop instance becomes slot 0
            idx[c, i, 0] = node
            idx[c, i, 1 : 1 + len(ss)] = ss
    return dict(perms=perms, D=D, idx=idx, n_buckets=n_buckets, npc=npc,
                npc_pad=npc_pad)


def _w_ext(W, a_s, a_d, heads, out_ch):
    F = heads * out_ch
    A_s = np.zeros((F, heads), np.float32)
    A_d = np.zeros((F, heads), np.float32)
    for h in range(heads):
        A_s[h * out_ch : (h + 1) * out_ch, h] = a_s[h]
        A_d[h * out_ch : (h + 1) * out_ch, h] = a_d[h]
    return np.concatenate([W, W @ A_s, W @ A_d], axis=1).astype(np.float32)


def _pack_idx(idx_tab, D, pad_val):
    """[npc_pad, Dmax] -> flat [sum 128*D_k] (bucket-major, partition-major)."""
    n_buckets = len(D)
    chunks = []
    for k in range(n_buckets):
        Dk = int(D[k])
        blk = idx_tab[k * P : (k + 1) * P, :Dk].copy()
        blk[blk < 0] = pad_val
        chunks.append(blk.reshape(-1))
    return np.concatenate(chunks).astype(np.int32)


# --------------------------------------------------------------------------
# device program
def _build_program(n_nodes, emb, hid, heads1, c1, repr_, D):
    F1 = hid                    # 128  (heads1*c1)
    H1 = heads1                 # 8
    C1 = c1                     # 16
    F2 = repr_                  # 64
    R1 = F1 + 2 * H1            # 144 table-1 row
    R2 = F2 + 2                 # 66  table-2 row
    n_buckets = len(D)
    npc_pad = n_buckets * P
    t2_rows = N_CORES * npc_pad + 1
    TOT = int(P * np.sum(D))

    nc = bass.Bass("TRN2", target_bir_lowering=False, debug=False,
                   num_devices=N_CORES)

    # x pre-expanded per slot (stream order [bucket][d][p]), transposed:
    # column j = x[src(slot j)] (zeros for padding slots)
    xe = nc.dram_tensor("xe", [emb, TOT], F32, kind="ExternalInput")
    msk = nc.dram_tensor("msk", [TOT], F32, kind="ExternalInput")
    w1e = nc.dram_tensor("w1e", [emb, R1], F32, kind="ExternalInput")
    w2e = nc.dram_tensor("w2e", [F1, R2], F32, kind="ExternalInput")
    b1b = nc.dram_tensor("b1b", [P, F1], F32, kind="ExternalInput")
    b2b = nc.dram_tensor("b2b", [P, F2], F32, kind="ExternalInput")
    pad2 = nc.dram_tensor("pad2", [1, R2], F32, kind="ExternalInput")
    idx2 = nc.dram_tensor("idx2", [TOT], I32, kind="ExternalInput")
    outp = nc.dram_tensor("out", [npc_pad, F2], F32, kind="ExternalOutput")
    if DEBUG:
        dbg_t2own = nc.dram_tensor("dbg_t2own", [npc_pad, R2], F32,
                                   kind="ExternalOutput")
        dbg_tbl2 = nc.dram_tensor("dbg_tbl2", [t2_rows, R2], F32,
                                  kind="ExternalOutput")
        dbg_e = nc.dram_tensor("dbg_e", [P, int(D[0]) * H1], F32,
                               kind="ExternalOutput")
        dbg_ee = nc.dram_tensor("dbg_ee", [P, int(D[0]) * H1], F32,
                                kind="ExternalOutput")
        dbg_s = nc.dram_tensor("dbg_s", [P, H1], F32, kind="ExternalOutput")
        dbg_agg = nc.dram_tensor("dbg_agg", [P, F1], F32, kind="ExternalOutput")
        dbg_o1 = nc.dram_tensor("dbg_o1", [P, F1], F32, kind="ExternalOutput")
        dbg_elu = nc.dram_tensor("dbg_elu", [P, F1], F32, kind="ExternalOutput")

    with tile.TileContext(nc) as tc:
        with (
            tc.tile_pool(name="const", bufs=1) as cpool,
            tc.tile_pool(name="mm", bufs=4) as mmpool,
            tc.tile_pool(name="psA", bufs=2, space="PSUM") as psA,
            tc.tile_pool(name="edge", bufs=3) as epool,
            tc.tile_pool(name="small", bufs=4) as spool,
            tc.tile_pool(name="dram", bufs=1, space="DRAM") as dpool,
        ):
            # ---- constants
            w1e_sb = cpool.tile([emb, R1], F32, tag="w1e")
            nc.sync.dma_start(w1e_sb[:], w1e[:])
            w2e_sb = cpool.tile([F1, R2], F32, tag="w2e")
            nc.sync.dma_start(w2e_sb[:], w2e[:])
            b1_sb = cpool.tile([P, F1], F32, tag="b1")
            nc.sync.dma_start(b1_sb[:], b1b[:])
            b2_sb = cpool.tile([P, F2], F32, tag="b2")
            nc.sync.dma_start(b2_sb[:], b2b[:])
            ident = cpool.tile([P, P], F32, tag="ident")
            from concourse.masks import make_identity
            make_identity(nc, ident[:])

            tbl2 = dpool.tile([t2_rows, R2], F32, tag="tbl2")
            t2own = dpool.tile([npc_pad, R2], F32, tag="t2own")

            # pad row of the layer-2 table
            p2_sb = cpool.tile([1, R2], F32, tag="p2")
            nc.sync.dma_start(p2_sb[:], pad2[:])
            nc.sync.dma_start(tbl2[t2_rows - 1 : t2_rows, :], p2_sb[:])

            # ---- phase B: layer-1 edge processing + local table-2 rows.
            # G rows arrive from dense matmuls over host-pre-expanded x
            # (aggregation commutes with W1, but we keep the full per-slot
            # row [h | a_s | a_d] so the attention math is unchanged).
            off = 0
            for k in range(n_buckets):
                Dk = int(D[k])
                G = epool.tile([P, Dk * R1], F32, tag="G")
                for d in range(Dk):
                    xt = mmpool.tile([P, P], F32, tag="xt")
                    nc.sync.dma_start(
                        xt[:], xe[:, (off + d * P) : (off + (d + 1) * P)])
                    ps = psA.tile([P, R1], F32, tag="psA")
                    nc.tensor.matmul(ps[:], lhsT=xt[:], rhs=w1e_sb[:],
                                     start=True, stop=True)
                    if d % 2 == 0:
                        nc.scalar.copy(G[:, d * R1 : (d + 1) * R1], ps[:])
                    else:
                        nc.vector.tensor_copy(G[:, d * R1 : (d + 1) * R1], ps[:])
                g3 = G[:].rearrange("p (d f) -> p d f", f=R1)

                mt = epool.tile([P, Dk], F32, tag="mt")
                nc.sync.dma_start(
                    mt[:], msk[off : off + P * Dk].rearrange("(d p) -> p d", p=P))

                e = epool.tile([P, Dk * H1], F32, tag="e")
                e3 = e[:].rearrange("p (d h) -> p d h", h=H1)
                nc.vector.tensor_tensor(
                    out=e3, in0=g3[:, :, F1 : F1 + H1],
                    in1=g3[:, 0:1, F1 + H1 : R1].to_broadcast([P, Dk, H1]),
                    op=OP.add)
                # padding slots: += -1e30 so they vanish in the softmax
                nc.vector.tensor_tensor(
                    out=e3, in0=e3,
                    in1=mt[:].rearrange("p (d o) -> p d o", o=1)
                        .to_broadcast([P, Dk, H1]),
                    op=OP.add)
                # leaky-relu(0.2) on DVE (HW Lrelu ignores alpha), clamp for pads
                ee = epool.tile([P, Dk * H1], F32, tag="ee")
                nc.vector.tensor_scalar(ee[:], e[:], 0.0, 0.2, op0=OP.min,
                                        op1=OP.mult)
                nc.vector.tensor_scalar_max(ee[:], ee[:], -80.0)
                nc.vector.scalar_tensor_tensor(out=e[:], in0=e[:], scalar=0.0,
                                               in1=ee[:], op0=OP.max, op1=OP.add)
                nc.scalar.activation(ee[:], e[:], ACTF.Exp)

                s = spool.tile([P, H1], F32, tag="s")
                nc.vector.tensor_reduce(
                    out=s[:], in_=ee[:].rearrange("p (d h) -> p h d", h=H1),
                    axis=AX.X, op=OP.add)
                s2 = spool.tile([P, H1], F32, tag="s2")
                nc.vector.tensor_scalar_add(s2[:], s[:], 1e-16)
                rs = spool.tile([P, H1], F32, tag="rs")
                nc.vector.reciprocal(rs[:], s2[:])

                prod = epool.tile([P, Dk * F1], F32, tag="prod")
                nc.vector.tensor_tensor(
                    out=prod[:].rearrange("p (d h c) -> p d h c", h=H1, c=C1),
                    in0=g3[:, :, 0:F1].rearrange("p d (h c) -> p d h c", c=C1),
                    in1=ee[:].rearrange("p (d h) -> p d h", h=H1)
                        .to_broadcast([P, Dk, H1, C1]),
                    op=OP.mult)
                agg = spool.tile([P, F1], F32, tag="agg")
                nc.vector.tensor_reduce(
                    out=agg[:], in_=prod[:].rearrange("p (d f) -> p f d", f=F1),
                    axis=AX.X, op=OP.add)

                o1 = spool.tile([P, F1], F32, tag="o1")
                nc.vector.tensor_tensor(
                    out=o1[:].rearrange("p (h c) -> p h c", h=H1),
                    in0=agg[:].rearrange("p (h c) -> p h c", h=H1),
                    in1=rs[:].rearrange("p (h o) -> p h o", o=1)
                        .to_broadcast([P, H1, C1]),
                    op=OP.mult)
                nc.vector.tensor_tensor(out=o1[:], in0=o1[:], in1=b1_sb[:],
                                        op=OP.add)
                # ELU = relu(x) + exp(min(x,0)) - 1
                xm = spool.tile([P, F1], F32, tag="xm")
                nc.vector.tensor_scalar_min(xm[:], o1[:], 0.0)
                em = spool.tile([P, F1], F32, tag="em")
                nc.scalar.activation(em[:], xm[:], ACTF.Exp)
                xr = spool.tile([P, F1], F32, tag="xr")
                nc.vector.tensor_scalar_max(xr[:], o1[:], 0.0)
                elu = spool.tile([P, F1], F32, tag="elu")
                nc.vector.scalar_tensor_tensor(
                    out=elu[:], in0=em[:], scalar=-1.0, in1=xr[:],
                    op0=OP.add, op1=OP.add)

                if DEBUG and k == 0:
                    nc.sync.dma_start(dbg_e[:, :], e[:])
                    nc.sync.dma_start(dbg_ee[:, :], ee[:])
                    nc.sync.dma_start(dbg_s[:, :], s[:])
                    nc.sync.dma_start(dbg_agg[:, :], agg[:])
                    nc.sync.dma_start(dbg_o1[:, :], o1[:])
                    nc.sync.dma_start(dbg_elu[:, :], elu[:])
                pst = psA.tile([P, P], F32, tag="psT")
                nc.tensor.transpose(pst[:], elu[:], ident[:])
                eluT = spool.tile([P, P], F32, tag="eluT")
                nc.scalar.copy(eluT[:], pst[:])
                ps2 = psA.tile([P, R2], F32, tag="psB")
                nc.tensor.matmul(ps2[:], lhsT=eluT[:], rhs=w2e_sb[:],
                                 start=True, stop=True)
                t2 = spool.tile([P, R2], F32, tag="t2")
                nc.scalar.copy(t2[:], ps2[:])
                nc.sync.dma_start(t2own[k * P : (k + 1) * P, :], t2[:])
                off += P * Dk

            # ---- phase C: AllGather layer-2 table
            nc.gpsimd.collective_compute(
                "AllGather", OP.bypass,
                replica_groups=[list(range(N_CORES))],
                ins=[t2own[:].opt()],
                outs=[tbl2[0 : t2_rows - 1, :].opt()])
            if DEBUG:
                nc.sync.dma_start(dbg_t2own[:, :], t2own[:])
                nc.sync.dma_start(dbg_tbl2[:, :], tbl2[:])

            # ---- phase D: layer-2 edge processing
            off = 0
            for k in range(n_buckets):
                Dk = int(D[k])
                it = epool.tile([P, Dk], I32, tag="idx2")
                nc.sync.dma_start(
                    it[:], idx2[off : off + P * Dk].rearrange("(p d) -> p d", p=P))
                G = epool.tile([P, Dk * R2], F32, tag="G2")
                for d in range(Dk):
                    nc.gpsimd.indirect_dma_start(
                        out=G[:, d * R2 : (d + 1) * R2], out_offset=None,
                        in_=tbl2[:],
                        in_offset=bass.IndirectOffsetOnAxis(ap=it[:, d : d + 1],
                                                            axis=0))
                g3 = G[:].rearrange("p (d f) -> p d f", f=R2)

                e = epool.tile([P, Dk], F32, tag="e2")
                nc.vector.tensor_tensor(
                    out=e[:].rearrange("p (d o) -> p d o", o=1),
                    in0=g3[:, :, F2 : F2 + 1],
                    in1=g3[:, 0:1, F2 + 1 : R2].to_broadcast([P, Dk, 1]),
                    op=OP.add)
                ee = epool.tile([P, Dk], F32, tag="ee2")
                nc.vector.tensor_scalar(ee[:], e[:], 0.0, 0.2, op0=OP.min,
                                        op1=OP.mult)
                nc.vector.tensor_scalar_max(ee[:], ee[:], -80.0)
                nc.vector.scalar_tensor_tensor(out=e[:], in0=e[:], scalar=0.0,
                                               in1=ee[:], op0=OP.max, op1=OP.add)
                nc.scalar.activation(ee[:], e[:], ACTF.Exp)

                s = spool.tile([P, 1], F32, tag="s_2")
                nc.vector.tensor_reduce(out=s[:], in_=ee[:], axis=AX.X, op=OP.add)
                s2 = spool.tile([P, 1], F32, tag="s2_2")
                nc.vector.tensor_scalar_add(s2[:], s[:], 1e-16)
                rs = spool.tile([P, 1], F32, tag="rs_2")
                nc.vector.reciprocal(rs[:], s2[:])

                prod = epool.tile([P, Dk * F2], F32, tag="prod2")
                nc.vector.tensor_tensor(
                    out=prod[:].rearrange("p (d c) -> p d c", c=F2),
                    in0=g3[:, :, 0:F2],
                    in1=ee[:].rearrange("p (d o) -> p d o", o=1)
                        .to_broadcast([P, Dk, F2]),
                    op=OP.mult)
                agg = spool.tile([P, F2], F32, tag="agg2")
                nc.vector.tensor_reduce(
                    out=agg[:], in_=prod[:].rearrange("p (d f) -> p f d", f=F2),
                    axis=AX.X, op=OP.add)

                o2 = spool.tile([P, F2], F32, tag="o2")
                nc.vector.tensor_scalar(o2[:], agg[:], rs[:, 0:1], None,
                                        op0=OP.mult)
                nc.vector.tensor_tensor(out=o2[:], in0=o2[:], in1=b2_sb[:],
                                        op=OP.add)
                nc.sync.dma_start(outp[k * P : (k + 1) * P, :], o2[:])
                off += P * Dk

    _split_waits(nc)
    return nc


# --------------------------------------------------------------------------
_CACHE = {}
LAST_PROGRAM = None  # (nc, in_maps) of the most recent kernel() call


def kernel(x, edge_index, W1, a1_src, a1_dst, b1, W2, a2_src, a2_dst, b2):
    x = np.asarray(x, np.float32)
    edge_index = np.asarray(edge_index)
    n_nodes, emb = x.shape
    heads1, c1 = np.asarray(a1_src).shape
    hid = heads1 * c1
    repr_ = np.asarray(W2).shape[1]

    ell = _build_ell(edge_index, n_nodes)
    D, n_buckets, npc_pad = ell["D"], ell["n_buckets"], ell["npc_pad"]

    w1e = _w_ext(np.asarray(W1, np.float32), np.asarray(a1_src, np.float32),
                 np.asarray(a1_dst, np.float32), heads1, c1)
    w2e = _w_ext(np.asarray(W2, np.float32), np.asarray(a2_src, np.float32),
                 np.asarray(a2_dst, np.float32), 1, repr_)

    R1 = hid + 2 * heads1
    R2 = repr_ + 2
    pad2 = np.zeros((1, R2), np.float32)
    pad2[0, repr_] = NEG_BIG

    # layer-2 index remap: global node id -> row in the allgathered table
    t2_rows = N_CORES * npc_pad + 1
    allperm = np.concatenate(ell["perms"])
    pos = np.full(n_nodes + 1, t2_rows - 1, dtype=np.int64)
    real = allperm >= 0
    pos[allperm[real]] = np.nonzero(real)[0]

    common = {
        "w1e": w1e, "w2e": w2e,
        "b1b": np.tile(np.asarray(b1, np.float32)[None, :], (P, 1)),
        "b2b": np.tile(np.asarray(b2, np.float32)[None, :], (P, 1)),
        "pad2": pad2,
    }
    x_pad = np.concatenate([x, np.zeros((1, emb), np.float32)])  # row for pads
    in_maps = []
    for c in range(N_CORES):
        idx_r = ell["idx"][c]
        # xe: [emb, TOT], slot stream [bucket][d][p]; zeros for padding slots
        xe_cols = []
        msk_parts = []
        for k in range(n_buckets):
            Dk = int(D[k])
            blk = idx_r[k * P : (k + 1) * P, :Dk]            # [128, Dk]
            ids = np.where(blk >= 0, blk, n_nodes).T          # [Dk, 128] (d-major)
            xe_cols.append(x_pad[ids.reshape(-1)])            # [(d p), emb]
            msk_parts.append(np.where(blk >= 0, 0.0, NEG_BIG).T.reshape(-1))
        xe = np.ascontiguousarray(np.concatenate(xe_cols).T)  # [emb, TOT]
        mskv = np.concatenate(msk_parts).astype(np.float32)
        i2_tab = np.where(idx_r >= 0, pos[np.maximum(idx_r, 0)], t2_rows - 1)
        i2 = _pack_idx(np.where(idx_r >= 0, i2_tab, -1), D, t2_rows - 1)
        in_maps.append({**common, "xe": xe, "msk": mskv, "idx2": i2})

    key = (n_nodes, emb, hid, heads1, c1, repr_, tuple(int(d) for d in D))
    if key not in _CACHE:
        _CACHE[key] = _build_program(n_nodes, emb, hid, heads1, c1, repr_, D)
    nc = _CACHE[key]
    global LAST_PROGRAM
    LAST_PROGRAM = (nc, in_maps)

    try:
        res = run_bass_kernel_spmd(nc, in_maps, core_ids=list(range(N_CORES)))
    except Exception:
        # transient device-state failures (e.g. a previous crashed process
        # left a core wedged) typically clear on retry
        import time as _time
        _time.sleep(10)
        res = run_bass_kernel_spmd(nc, in_maps, core_ids=list(range(N_CORES)))

    out = np.zeros((n_nodes, repr_), np.float32)
    for c in range(N_CORES):
        o = res.results[c]["out"]
        perm = ell["perms"][c]
        m = perm >= 0
        out[perm[m]] = o[m]
    return out
